# revision 25
# baseline (speedup 1.0000x reference)
"""DeepseekMoE layer on 8 Trainium2 NeuronCores (Bass/Tile, expert-parallel).

Sharding (per the expert-parallel hint):
  - 16 routed experts -> 2 per core, LOAD-BALANCED: experts are sorted by
    routed-token count and the i-th largest is paired with the i-th smallest,
    so slot 0 holds a "big" expert (C1 token slots) and slot 1 a "small" one
    (C2 slots).  Token dispatch (all-to-all) is emulated at the sharding
    layer: the host computes the discrete top-4 routing, gathers each
    expert's tokens into a compact transposed batch, and scatter-adds the
    compact expert outputs back into the full output ("combine").
  - Shared expert is tensor-parallel over its intermediate dim (2816/8 = 352
    columns per core); the 8 partial outputs are summed on gather.
  - Gate (softmax + renormalized top-4 combine weights) is replicated and
    computed ON DEVICE from the hidden states; the host only supplies the
    discrete 0/1 top-4 mask (routing decision) and gather indices.

All FLOPs that produce output values run on device.  Matmuls use bfloat16
operands with fp32 PSUM accumulation: on TRN2 the PE runs bf16 at the same
1 cycle/row as float32r (moving>=256), so this costs no compute time, but it
halves HBM traffic -- and the 16 shared DMA engines (360 GB/s aggregate) are
the baseline bottleneck (~105 MB/core in fp32).  Expert outputs and shared
partials return in bf16 as well; the host combine accumulates in float64.

Weights are host-packed into stationary-tile-major layout ([m-tile, partition,
k-tile, col]) so each m-column's whole contraction loads as one DMA with
multi-KB descriptors: the Sync engine costs ~620ns per DMA instruction
regardless of size, so small-tile DMA streams are issue-bound.
"""

import os
import numpy as np
from ml_dtypes import bfloat16

H = 2048          # hidden size
E = 16            # routed experts
TOPK = 4
I = 1408          # routed expert intermediate
ISH = 2816        # shared expert intermediate
T = 1024          # tokens
P = 128
NCORES = 8
EPC = 2           # experts per core
ISS = ISH // NCORES                  # 352 shared columns per core
ISSP = 384                           # padded to 3 full 128-tiles
KH = H // P                          # 16 k-tiles over H
MI = I // P                          # 11 m-tiles over I
MH = H // P                          # 16 m-tiles over H
KI = I // P                          # 11 k-tiles over I
KS = ISSP // P                       # 3 k-tiles over padded shared slice
ZERO_ROW_FLAT = T * E                # flat index of the zeroed scratch row

_NC_CACHE = {}
LAST_RESULTS = None  # BassKernelResults of the most recent run (for test.py)


def _token_chunks(C):
    """Split [0, C) into matmul moving-dim chunks of <=512."""
    out = []
    off = 0
    while off < C:
        sz = min(512, C - off)
        out.append((off, sz))
        off += sz
    return out


def _pack_st(w, KT, MT):
    """[KT*P, MT*P] -> [MT*P, KT*P] tile-major stationary pack (bf16).

    packed[m*P + p, k*P + c] = w[k*P + p, m*P + c], so the device loads
    rows [m*P, (m+1)*P) as one [P, KT*P] block whose column-slice k is the
    stationary tile for (k, m).
    """
    return np.ascontiguousarray(
        w.reshape(KT, P, MT, P).transpose(2, 1, 0, 3).reshape(MT * P, KT * P)
    ).astype(bfloat16)


def _build(Cs):
    import concourse.bacc as bacc
    import concourse.bass as bass
    import concourse.mybir as mybir
    import concourse.tile as tile
    from concourse.masks import make_identity

    f32 = mybir.dt.float32
    bf16 = mybir.dt.bfloat16
    f8 = mybir.dt.float8e3
    i32 = mybir.dt.int32
    SILU = mybir.ActivationFunctionType.Silu
    EXP = mybir.ActivationFunctionType.Exp
    X = mybir.AxisListType.X

    C1, C2 = Cs
    CTOT = C1 + C2
    COFF = [0, C1]                       # slot offsets into widx/zt
    CHS = [_token_chunks(C1), _token_chunks(C2)]
    CMAX = max(C1, C2)
    NT = T // 512     # token chunks for shared/gate (2)

    nc = bacc.Bacc("TRN2", target_bir_lowering=False, debug=False)

    xt_h = nc.dram_tensor("xt", [H, T], bf16, kind="ExternalInput")
    gwtb_h = nc.dram_tensor("gwtb", [P, KH * E], bf16, kind="ExternalInput")
    maskb_h = nc.dram_tensor("maskb", [P, (T // P) * E], f32, kind="ExternalInput")
    xg_h = [nc.dram_tensor(f"xg{j}", [P, KH * Cs[j]], bf16, kind="ExternalInput") for j in range(EPC)]
    widx_h = nc.dram_tensor("widx", [CTOT, 1], i32, kind="ExternalInput")
    wg_h = [nc.dram_tensor(f"wg{j}", [I, H], bf16, kind="ExternalInput") for j in range(EPC)]
    wu_h = [nc.dram_tensor(f"wu{j}", [I, H], bf16, kind="ExternalInput") for j in range(EPC)]
    wd_h = [nc.dram_tensor(f"wd{j}", [H, I], bf16, kind="ExternalInput") for j in range(EPC)]
    swg_h = nc.dram_tensor("swg", [ISSP, H], bf16, kind="ExternalInput")
    swu_h = nc.dram_tensor("swu", [ISSP, H], bf16, kind="ExternalInput")
    swd_h = nc.dram_tensor("swd", [H, ISSP], bf16, kind="ExternalInput")
    zt_h = nc.dram_tensor("zt", [H, CTOT], bf16, kind="ExternalOutput")
    st_h = nc.dram_tensor("st", [H, T], bf16, kind="ExternalOutput")

    with tile.TileContext(nc) as tc:
        with (
            tc.tile_pool(name="resident", bufs=1) as res_pool,
            tc.tile_pool(name="xgp", bufs=1) as xg_pool,
            tc.tile_pool(name="acts", bufs=1) as act_pool,
            tc.tile_pool(name="wstream", bufs=3) as wst_pool,
            tc.tile_pool(name="dstream", bufs=3) as dst_pool,
            tc.tile_pool(name="sstream", bufs=2) as sst_pool,
            tc.tile_pool(name="small", bufs=2) as small_pool,
            tc.tile_pool(name="stage", bufs=3) as stage_pool,
            tc.tile_pool(name="ps", bufs=1, space="PSUM") as ps_pool,
            tc.tile_pool(name="dram", bufs=1, space="DRAM") as dram_pool,
        ):
            # ---------------- resident loads ----------------
            # Startup critical path: the first routed matmul needs only
            # xgb0's first quarter + the first weight tiles on the sync
            # queue; xt streams concurrently on the scalar engine's HWDGE
            # queue and the gate's logit matmuls are interleaved into
            # upgate(0)'s m-loop as xt tiles land.  The softmax + indirect
            # combine-weight gather issue mid-upgate(0) so their ~15us
            # latency chain finishes long before down(0) consumes wb.
            xgb = [xg_pool.tile([P, KH * Cs[j]], bf16, name=f"xgb{j}", tag=f"xgb{j}") for j in range(EPC)]
            gwtb = res_pool.tile([P, KH * E], bf16, name="gwtb", tag="gwtb")
            nc.gpsimd.dma_start(gwtb[:], gwtb_h[:])
            maskb = res_pool.tile([P, (T // P) * E], f32, name="maskb", tag="maskb")
            nc.gpsimd.dma_start(maskb[:], maskb_h[:])
            q0 = KH // 4 * Cs[0]
            nc.sync.dma_start(xgb[0][:, :q0], xg_h[0][:, :q0])
            xt2 = [res_pool.tile([P, 2 * T], bf16, name=f"xt2_{kk}", tag=f"xt2_{kk}") for kk in range(KH // 2)]

            def load_xg0_rest():
                q0 = KH // 4 * Cs[0]
                for q in range(1, 4):
                    nc.sync.dma_start(xgb[0][:, q * q0:(q + 1) * q0], xg_h[0][:, q * q0:(q + 1) * q0])

            def load_xt(kk):
                nc.scalar.dma_start(
                    xt2[kk][:].rearrange("p (a t) -> p a t", a=2),
                    xt_h[kk * 2 * P:(kk + 1) * 2 * P, :].rearrange("(a p) t -> p a t", p=P))

            def load_xg1():
                h1 = KH // 2 * Cs[1]
                nc.sync.dma_start(xgb[1][:, :h1], xg_h[1][:, :h1])
                nc.sync.dma_start(xgb[1][:, h1:], xg_h[1][:, h1:])

            xt_t = [xt2[k // 2][:, (k % 2) * T:(k % 2 + 1) * T] for k in range(KH)]
            ident = res_pool.tile([P, P], f32, name="ident", tag="ident")
            make_identity(nc, ident[:])
            zbias = res_pool.tile([P, 1], f32, name="zbias", tag="zbias")
            nc.vector.memset(zbias[:], 0.0)

            # combine-weight scratch in HBM: rows 0..T-1 = combine, row T = zeros
            wflat = dram_pool.tile([(T + 1) * E, 1], f32, name="wflat")
            wflat2d = wflat[:].rearrange("(a b) o -> a (b o)", b=E)
            zrow = res_pool.tile([1, E], f32, name="zrow", tag="zrow")
            nc.vector.memset(zrow[:], 0.0)
            nc.gpsimd.dma_start(wflat2d[T:T + 1, :], zrow[:])

            wb = [res_pool.tile([P, Cs[j]], f32, name=f"wb{j}", tag=f"wb{j}") for j in range(EPC)]
            a_t = [[act_pool.tile([P, Cs[j]], bf16, name=f"a{j}_{m}", tag=f"a{j}_{m}") for m in range(MI)]
                   for j in range(EPC)]
            sg_t = [act_pool.tile([P, T], f32, name=f"sg{m}", tag="sgtmp", bufs=2) for m in range(KS)]
            as_t = [act_pool.tile([P, T], bf16, name=f"as{m}", tag=f"as{m}") for m in range(KS)]

            # ---------------- emission sections ----------------
            lgps_box = []

            def emit_gate_mm(k2):
                # two k-steps of the gate logits accumulation (consumes xt2[k2])
                if not lgps_box:
                    lgps_box.append(ps_pool.tile([E, T], f32, name="lgps", tag="B1", bufs=2))
                lgps = lgps_box[0]
                for k in (2 * k2, 2 * k2 + 1):
                    for n in range(NT):
                        nc.tensor.matmul(
                            lgps[:, n * 512:(n + 1) * 512],
                            lhsT=gwtb[:, k * E:(k + 1) * E],
                            rhs=xt_t[k][:, n * 512:(n + 1) * 512],
                            start=(k == 0), stop=(k == KH - 1),
                        )

            def emit_gate_post():
                lgps = lgps_box[0]
                lgsb = res_pool.tile([E, T], f32, name="lgsb", tag="lgsb")
                nc.scalar.copy(lgsb[:], lgps[:])
                for t8 in range(T // P):
                    trps = ps_pool.tile([P, E], f32, name=f"tr{t8}", tag="A1", bufs=4)
                    nc.tensor.transpose(
                        out=trps[:], in_=lgsb[:, t8 * P:(t8 + 1) * P], identity=ident[0:E, 0:E],
                    )
                    sc = small_pool.tile([P, E], f32, name=f"sc{t8}", tag="sc")
                    nc.scalar.activation(sc[:], trps[:], EXP, bias=zbias[:])
                    mskd = small_pool.tile([P, E], f32, name=f"mskd{t8}", tag="mskd")
                    nc.vector.tensor_mul(out=mskd[:], in0=sc[:], in1=maskb[:, t8 * E:(t8 + 1) * E])
                    ssum = small_pool.tile([P, 1], f32, name=f"ssum{t8}", tag="ssum")
                    nc.vector.reduce_sum(ssum[:], mskd[:], axis=X)
                    rsum = small_pool.tile([P, 1], f32, name=f"rsum{t8}", tag="rsum")
                    nc.vector.reciprocal(rsum[:], ssum[:])
                    comb = small_pool.tile([P, E], f32, name=f"comb{t8}", tag="comb")
                    nc.vector.tensor_scalar_mul(comb[:], mskd[:], rsum[:, :1])
                    nc.gpsimd.dma_start(wflat2d[t8 * P:(t8 + 1) * P, :], comb[:])

            def _chunks(j):
                out, off = [], 0
                while off < Cs[j]:
                    out.append((off, min(P, Cs[j] - off)))
                    off += P
                return out

            wslots = {}
            its = {}

            def emit_gather_idx(j):
                # index loads have no deps -- issue at kernel start
                for (off, csz) in _chunks(j):
                    it = res_pool.tile([P, 1], i32, name=f"it{j}_{off}", tag=f"it{j}_{off}")
                    nc.gpsimd.dma_start(it[:csz], widx_h[COFF[j] + off:COFF[j] + off + csz, :])
                    its[(j, off)] = it

            def emit_gather_dma(j):
                # indirect gather of per-slot combine weights (latency hidden
                # under routed matmuls; PE-side broadcast happens later)
                for (off, csz) in _chunks(j):
                    it = its[(j, off)]
                    wslot = res_pool.tile([P, 1], f32, name=f"ws{j}_{off}", tag=f"ws{j}_{off}")
                    nc.gpsimd.indirect_dma_start(
                        out=wslot[:csz, :], out_offset=None, in_=wflat[:],
                        in_offset=bass.IndirectOffsetOnAxis(ap=it[:csz, :1], axis=0),
                    )
                    wslots[(j, off)] = wslot

            def emit_gather_pe(j):
                # wslot [csz,1] -> partition-broadcast wb[j][:, off:off+csz]
                for (off, csz) in _chunks(j):
                    wslot = wslots[(j, off)]
                    wbps = ps_pool.tile([P, P], f32, name=f"wbps{j}_{off}", tag="A1", bufs=4)
                    nc.tensor.transpose(
                        out=wbps[:, :csz],
                        in_=wslot[:csz, :1].to_broadcast([csz, P]),
                        identity=ident[0:csz, 0:csz],
                    )
                    nc.vector.tensor_copy(wb[j][:, off:off + csz], wbps[:, :csz])

            def emit_upgate(j, weave=None, interleave=None):
                Cj = Cs[j]
                xg_t = [xgb[j][:, k * Cj:(k + 1) * Cj] for k in range(KH)]
                g_t = [act_pool.tile([P, Cj], f32, name=f"g{j}_{m}", tag="gtmp", bufs=3) for m in range(MI)]
                for m in range(MI):
                    wgb1 = wst_pool.tile([P, KH * P], bf16, name=f"wgb{j}_{m}", tag="wblk", bufs=8)
                    nc.sync.dma_start(wgb1[:], wg_h[j][m * P:(m + 1) * P, :])
                    wub1 = wst_pool.tile([P, KH * P], bf16, name=f"wub{j}_{m}", tag="wblk", bufs=8)
                    nc.sync.dma_start(wub1[:], wu_h[j][m * P:(m + 1) * P, :])
                    if weave is not None and m < len(weave):
                        weave[m]()
                    wgb = [wgb1[:, :8 * P], wgb1[:, 8 * P:]]
                    wub = [wub1[:, :8 * P], wub1[:, 8 * P:]]
                    for (coff, csz) in CHS[j]:
                        psg = ps_pool.tile([P, csz], f32, name=f"psg{j}_{m}_{coff}", tag="A1", bufs=4)
                        for k in range(KH):
                            nc.tensor.matmul(psg[:], lhsT=wgb[k // 8][:, (k % 8) * P:(k % 8 + 1) * P],
                                             rhs=xg_t[k][:, coff:coff + csz],
                                             start=(k == 0), stop=(k == KH - 1))
                        nc.scalar.activation(g_t[m][:, coff:coff + csz], psg[:], SILU, bias=zbias[:])
                        psu = ps_pool.tile([P, csz], f32, name=f"psu{j}_{m}_{coff}", tag="A1", bufs=4)
                        for k in range(KH):
                            nc.tensor.matmul(psu[:], lhsT=wub[k // 8][:, (k % 8) * P:(k % 8 + 1) * P],
                                             rhs=xg_t[k][:, coff:coff + csz],
                                             start=(k == 0), stop=(k == KH - 1))
                        # a = silu(g) * u straight out of PSUM, rounded to bf16
                        nc.vector.tensor_mul(out=a_t[j][m][:, coff:coff + csz],
                                             in0=g_t[m][:, coff:coff + csz], in1=psu[:])
                    if interleave is not None:
                        interleave(m)

            def emit_down(j, interleave=None):
                for m in range(MH):
                    if interleave is not None and m % 2 == 0:
                        interleave(m // 2)
                    wdb1 = dst_pool.tile([P, KI * P], bf16, name=f"wdb{j}_{m}", tag="wdb", bufs=8)
                    nc.sync.dma_start(wdb1[:], wd_h[j][m * P:(m + 1) * P, :])
                    for (coff, csz) in CHS[j]:
                        psz = ps_pool.tile([P, csz], f32, name=f"psz{j}_{m}_{coff}", tag="A1", bufs=4)
                        for k in range(KI):
                            nc.tensor.matmul(psz[:], lhsT=wdb1[:, k * P:(k + 1) * P],
                                             rhs=a_t[j][k][:, coff:coff + csz],
                                             start=(k == 0), stop=(k == KI - 1))
                        zst = stage_pool.tile([P, csz], bf16, name=f"zst{j}_{m}_{coff}", tag="zst", bufs=2)
                        # combine-weight scaling fused into the eviction
                        nc.vector.tensor_mul(out=zst[:], in0=wb[j][:, coff:coff + csz], in1=psz[:])
                        nc.gpsimd.dma_start(zt_h[m * P:(m + 1) * P, COFF[j] + coff:COFF[j] + coff + csz], zst[:])

            def emit_shared_ug(mi, weave=None):
                sgb1 = sst_pool.tile([P, KH * P], bf16, name=f"sgb{mi}", tag="ssb", bufs=4)
                nc.sync.dma_start(sgb1[:], swg_h[mi * P:(mi + 1) * P, :])
                if weave is not None:
                    for w in weave:
                        w()
                sgb = [sgb1[:, :8 * P], sgb1[:, 8 * P:]]
                psgs = ps_pool.tile([P, T], f32, name=f"psgs{mi}", tag="B1", bufs=2)
                for k in range(KH):
                    for n in range(NT):
                        nc.tensor.matmul(psgs[:, n * 512:(n + 1) * 512],
                                         lhsT=sgb[k // 8][:, (k % 8) * P:(k % 8 + 1) * P],
                                         rhs=xt_t[k][:, n * 512:(n + 1) * 512],
                                         start=(k == 0), stop=(k == KH - 1))
                nc.scalar.activation(sg_t[mi][:], psgs[:], SILU, bias=zbias[:])
                sub1 = sst_pool.tile([P, KH * P], bf16, name=f"sub{mi}", tag="ssb", bufs=4)
                nc.sync.dma_start(sub1[:], swu_h[mi * P:(mi + 1) * P, :])
                sub = [sub1[:, :8 * P], sub1[:, 8 * P:]]
                psus = ps_pool.tile([P, T], f32, name=f"psus{mi}", tag="B1", bufs=2)
                for k in range(KH):
                    for n in range(NT):
                        nc.tensor.matmul(psus[:, n * 512:(n + 1) * 512],
                                         lhsT=sub[k // 8][:, (k % 8) * P:(k % 8 + 1) * P],
                                         rhs=xt_t[k][:, n * 512:(n + 1) * 512],
                                         start=(k == 0), stop=(k == KH - 1))
                nc.vector.tensor_mul(out=as_t[mi][:], in0=sg_t[mi][:], in1=psus[:])

            def emit_shared_down(ms):
                for m in ms:
                    sdb = sst_pool.tile([P, KS * P], bf16, name=f"sdb{m}", tag="sdb", bufs=4)
                    nc.gpsimd.dma_start(sdb[:], swd_h[m * P:(m + 1) * P, :])
                    psys = ps_pool.tile([P, T], f32, name=f"psys{m}", tag="B1", bufs=2)
                    for ki in range(KS):
                        for n in range(NT):
                            nc.tensor.matmul(psys[:, n * 512:(n + 1) * 512],
                                             lhsT=sdb[:, ki * P:(ki + 1) * P],
                                             rhs=as_t[ki][:, n * 512:(n + 1) * 512],
                                             start=(ki == 0), stop=(ki == KS - 1))
                    sstg = stage_pool.tile([P, T], bf16, name=f"sstg{m}", tag="sstage", bufs=2)
                    nc.scalar.copy(sstg[:], psys[:])
                    nc.sync.dma_start(st_h[m * P:(m + 1) * P, :], sstg[:])

            # Routed expert 0 starts immediately (needs only xgb0 + first
            # weight tiles); the gate's logit matmuls interleave into its
            # m-loop, consuming xt tiles as they stream in behind the weights.
            # The gather's indirect DMAs issue right after the softmax so
            # their ~20us latency chain hides under the remaining matmuls;
            # only the cheap PE-side broadcasts run just before each down.
            def up0_hook(m):
                if 2 <= m <= 5:
                    emit_gate_mm(2 * (m - 2))
                    emit_gate_mm(2 * (m - 2) + 1)
                if m == 5:
                    emit_gather_idx(0)
                    emit_gather_idx(1)
                elif m == 6:
                    emit_gate_post()
                    emit_gather_dma(0)
                    emit_gather_dma(1)
                elif m == 8:
                    load_xg1()

            for kk in range(KH // 2):
                load_xt(kk)
            emit_upgate(0, weave=[load_xg0_rest], interleave=up0_hook)
            emit_shared_ug(0)
            emit_gather_pe(0)
            emit_down(0)
            emit_shared_ug(1)
            emit_upgate(1)
            emit_shared_ug(2)
            emit_gather_pe(1)
            emit_down(1)
            emit_shared_down(list(range(MH)))

    nc.compile()
    return nc


def _get_nc(Cs):
    if Cs not in _NC_CACHE:
        _NC_CACHE[Cs] = _build(Cs)
    return _NC_CACHE[Cs]


def _pad32(n):
    return max(32, int(np.ceil(n / 32)) * 32)


def kernel(**inputs):
    global LAST_RESULTS
    from concourse.bass_utils import run_bass_kernel_spmd

    hs = np.asarray(inputs["hidden_states"], dtype=np.float32)
    gate_w = np.asarray(inputs["gate_w"], dtype=np.float32)
    w_gate = np.asarray(inputs["w_gate"], dtype=np.float32)
    w_up = np.asarray(inputs["w_up"], dtype=np.float32)
    w_down = np.asarray(inputs["w_down"], dtype=np.float32)
    sw_gate = np.asarray(inputs["sw_gate"], dtype=np.float32)
    sw_up = np.asarray(inputs["sw_up"], dtype=np.float32)
    sw_down = np.asarray(inputs["sw_down"], dtype=np.float32)

    orig_shape = hs.shape
    x = hs.reshape(-1, H)
    assert x.shape[0] == T

    # ---- host: discrete routing only (top-4 selection + dispatch tables) ----
    logits = x @ gate_w.T
    smax = logits.max(axis=-1, keepdims=True)
    sc = np.exp(logits - smax)
    sc /= sc.sum(axis=-1, keepdims=True)
    order = np.argsort(-sc, axis=-1, kind="stable")[:, :TOPK]
    mask = np.zeros((T, E), dtype=np.float32)
    mask[np.arange(T)[:, None], order] = 1.0
    tok_lists = [np.nonzero(mask[:, e])[0].astype(np.int64) for e in range(E)]
    counts = np.array([len(tk) for tk in tok_lists])

    # balanced pairing: i-th largest with i-th smallest; slot 0 = bigger.
    # Slot widths are the exact max count per slot (moving dims need no
    # alignment; the cache key is just (C1, C2)).
    o = np.argsort(-counts, kind="stable")
    core_exp = [(int(o[i]), int(o[2 * NCORES - 1 - i])) for i in range(NCORES)]
    C1 = max(32, max(counts[p[0]] for p in core_exp))
    C2 = max(32, max(counts[p[1]] for p in core_exp))
    Cs = (C1, C2)

    nc = _get_nc(Cs)

    xTb = np.ascontiguousarray(x.T).astype(bfloat16)
    # gate weights packed: gwtb[p, k*E + e] = gate_w[e, k*P + p]
    gwtb = np.ascontiguousarray(
        gate_w.T.reshape(KH, P, E).transpose(1, 0, 2).reshape(P, KH * E)).astype(bfloat16)
    # mask packed: maskb[p, t8*E + e] = mask[t8*P + p, e]
    maskb = np.ascontiguousarray(mask.reshape(T // P, P, E).transpose(1, 0, 2).reshape(P, (T // P) * E))

    # shared slices, zero-padded to 384 and tile-major packed
    def pad_cols(w, newc):
        out = np.zeros((w.shape[0], newc), dtype=np.float32)
        out[:, :w.shape[1]] = w
        return out

    def pad_rows(w, newr):
        out = np.zeros((newr, w.shape[1]), dtype=np.float32)
        out[:w.shape[0], :] = w
        return out

    in_maps = []
    for c in range(NCORES):
        es = core_exp[c]
        widx = np.full((C1 + C2, 1), ZERO_ROW_FLAT, dtype=np.int32)
        im = {
            "xt": xTb, "gwtb": gwtb, "maskb": maskb, "widx": widx,
            "swg": _pack_st(pad_cols(sw_gate[:, c * ISS:(c + 1) * ISS], ISSP), KH, KS),
            "swu": _pack_st(pad_cols(sw_up[:, c * ISS:(c + 1) * ISS], ISSP), KH, KS),
            "swd": _pack_st(pad_rows(sw_down[c * ISS:(c + 1) * ISS, :], ISSP), KS, MH),
        }
        for j, e in enumerate(es):
            Cj = Cs[j]
            tk = tok_lists[e]
            widx[Cs[0] * j:Cs[0] * j + len(tk), 0] = (tk * E + e).astype(np.int32)
            # gathered activations, tile-major: xg[p, k*C + c] = x[tok_c, k*P + p]
            xg = np.zeros((P, KH * Cj), dtype=bfloat16)
            g = xTb[:, tk].reshape(KH, P, len(tk)).transpose(1, 0, 2)  # [P, KH, n]
            xg.reshape(P, KH, Cj)[:, :, :len(tk)] = g
            im[f"xg{j}"] = xg
            im[f"wg{j}"] = _pack_st(w_gate[e], KH, MI)
            im[f"wu{j}"] = _pack_st(w_up[e], KH, MI)
            im[f"wd{j}"] = _pack_st(w_down[e], KI, MH)
        in_maps.append(im)

    trace = bool(int(os.environ.get("BASSMOE_TRACE", "0")))
    kwargs = {}
    if trace:
        kwargs = dict(trace=True, tmpdir=os.environ.get("BASSMOE_TRACE_DIR") or None)
        tcores = os.environ.get("BASSMOE_TRACE_CORES")
        if tcores:
            kwargs["trace_cores"] = [int(x) for x in tcores.split(",")]
            kwargs["stitch_traces"] = False
    res = run_bass_kernel_spmd(nc, in_maps, core_ids=list(range(NCORES)), **kwargs)
    LAST_RESULTS = res

    # ---- host: unshard (scatter-add compact expert outputs + sum partials) ----
    y = np.zeros((T, H), dtype=np.float64)
    st_sum = np.zeros((H, T), dtype=np.float64)
    for c in range(NCORES):
        r = res.results[c]
        st_sum += r["st"].astype(np.float64)
        for j, e in enumerate(core_exp[c]):
            tk = tok_lists[e]
            y[tk] += r["zt"][:, COFF_SLOT(Cs, j) : COFF_SLOT(Cs, j) + len(tk)].astype(np.float64).T
    y += st_sum.T
    return y.astype(np.float32).reshape(orig_shape)


def COFF_SLOT(Cs, j):
    return 0 if j == 0 else Cs[0]
